# revision 9
# baseline (speedup 1.0000x reference)
"""Trainium2 Bass kernel for nn_Attention_65747359367242.

Per-batch tanh-attention with head-mean:
  Q = x@Wq+bq, K = cond@Wk+bk, V = cond@Wv+bv   (4 heads of 32 dims)
  S_h = Q_h K_h^T / sqrt(128)
  A   = mean_h tanh(mask + S_h)
  out = A @ V

Sharding: pure data-parallel, batch b -> core b (B=8, 8 cores). No collectives.

Device strategy per core (transposed orientation: scores S^T[m, n]):
  - host feeds x^T, cond^T, mask^T (bf16) + prescaled weights
  - Q^T/K^T/V computed on device via small matmuls (biases added as rank-1
    matmuls accumulating into the same PSUM)
  - main loop over (ncg: 4 n-chunks of 512) x (mt: 16 m-tiles of 128):
      * mask^T tile injected into 4 PSUM half-banks (one per head) via
        full-array identity matmuls (start=True clears, sets has_written)
      * 16 score matmuls (4 heads x 4 m-subtiles, K=32) packed at the 16
        32x32 tile positions accumulate S_h^T on top -> PSUM = mask + S_h
      * ScalarE tanh PSUM -> SBUF bf16, one per head-PAIR (FD=1024)
      * head-mean folded into AV by linearity: av[d, n] accumulates one
        matmul per head, moving operand = tanh slice (V' = Wv/4 prescaled)
  - out^T streamed to DRAM; host transposes back.

The ScalarE tanh stream (~128 us/core) is the theoretical bottleneck;
everything else (PE ~75 us, DVE ~25 us, DMA ~25 us) pipelines underneath.
"""

import math
import os
import sys

import numpy as np

sys.path.insert(0, "/opt/trn_rl_repo")

KREP = int(os.environ.get("KREP", "1"))  # on-device repeats of main loop

B, N, D = 8, 2048, 128
H, DH = 4, 32
NCH = 512            # n-chunk (free dim of score tiles / psum bank)
N_NCH = N // NCH     # 4
N_MT = N // 128      # 16 m-tiles

_NC_CACHE = {}


def _build_nc():
    from concourse import bass, tile
    from concourse.tile import add_dep_helper

    mybir = sys.modules["concourse.mybir"]
    f32 = mybir.dt.float32
    bf16 = mybir.dt.bfloat16
    TANH = mybir.ActivationFunctionType.Tanh

    nc = bass.Bass()

    xT = nc.declare_dram_parameter("xT", [D, N], bf16, isOutput=False)
    condT = nc.declare_dram_parameter("condT", [D, N], bf16, isOutput=False)
    maskT = nc.declare_dram_parameter("maskT", [N, N], bf16, isOutput=False)
    Wq = nc.declare_dram_parameter("Wq", [D, D], bf16, isOutput=False)
    Wk = nc.declare_dram_parameter("Wk", [D, D], bf16, isOutput=False)
    Wv4 = nc.declare_dram_parameter("Wv4", [D, D], bf16, isOutput=False)
    bq = nc.declare_dram_parameter("bq", [D, D], bf16, isOutput=False)
    bk = nc.declare_dram_parameter("bk", [D, D], bf16, isOutput=False)
    bv4 = nc.declare_dram_parameter("bv4", [D, D], bf16, isOutput=False)
    onesm = nc.declare_dram_parameter("onesm", [D, NCH], bf16, isOutput=False)
    eyef = nc.declare_dram_parameter("eyef", [D, D], bf16, isOutput=False)
    outT = [nc.declare_dram_parameter(f"outT{i}", [D, NCH], f32,
                                      isOutput=True) for i in range(N_NCH)]

    with tile.TileContext(nc) as tc:
        with (
            tc.tile_pool(name="const", bufs=1) as cpool,
            tc.tile_pool(name="proj", bufs=1) as projpool,
            tc.tile_pool(name="mask", bufs=16) as mpool,
            tc.tile_pool(name="th", bufs=6) as thpool,
            tc.tile_pool(name="osb", bufs=4 * KREP) as opool,
            tc.tile_pool(name="ps", bufs=3, space="PSUM") as pspool,
            tc.tile_pool(name="av", bufs=2, space="PSUM") as avpool,
            tc.tile_pool(name="gsb", bufs=66 * KREP) as gsbpool,
        ):
            # ---- load constants / inputs ----
            wq_sb = cpool.tile([D, D], bf16, tag="wq")
            wk_sb = cpool.tile([D, D], bf16, tag="wk")
            wv_sb = cpool.tile([D, D], bf16, tag="wv")
            bq_sb = cpool.tile([D, D], bf16, tag="bq")
            bk_sb = cpool.tile([D, D], bf16, tag="bk")
            bv_sb = cpool.tile([D, D], bf16, tag="bv")
            ones_sb = cpool.tile([D, NCH], bf16, tag="ones")
            eyef_sb = cpool.tile([D, D], bf16, tag="eyef")
            xT_sb = cpool.tile([D, N], bf16, tag="xT")
            condT_sb = cpool.tile([D, N], bf16, tag="condT")

            # ldweights gates absorb DMA waits on the PE side (the Matmult
            # HW struct fits only one sync wait). They must be FULL-HEIGHT
            # [128, 1] loads: partial-height standalone ldweights before
            # tile_position matmuls hard-fault the PE
            # (NRT_EXEC_UNIT_UNRECOVERABLE).
            for sb_t, dr_t in [(wq_sb, Wq), (wk_sb, Wk), (wv_sb, Wv4),
                               (eyef_sb, eyef), (xT_sb, xT),
                               (condT_sb, condT)]:
                nc.sync.dma_start(out=sb_t[:], in_=dr_t[:])
                nc.tensor.ldweights(sb_t[:, 0:1])
            for sb_t, dr_t in [(bq_sb, bq), (bk_sb, bk), (bv_sb, bv4),
                               (ones_sb, onesm)]:
                nc.sync.dma_start(out=sb_t[:], in_=dr_t[:])
                nc.tensor.ldweights(sb_t[:, 0:1])

            # ---- mask prefetch: 16 full-row DMAs [128, 2048] (4 KB/line) ----
            mk_tiles = []
            tail_insts = []
            for mt in range(N_MT):
                mk = mpool.tile([128, N], bf16, tag="mk",
                                name=f"mk_{mt}", bufs=16)
                dmi = nc.sync.dma_start(
                    out=mk[:], in_=maskT[mt * 128:(mt + 1) * 128, :])
                # gate: absorbs the DMA wait so inject matmuls carry only
                # their slot-release wait
                nc.tensor.ldweights(mk[:, 0:1])
                mk_tiles.append(mk)
                if mt >= N_MT - 8:
                    # predrain nops (below) absorb these lanes' final DMAHW
                    # ticks so the kernel-tail drain fits its wait slots
                    tail_insts.append(dmi)

            # ---- projections ----
            # Q^T[d, n] = Wq'^T x^T + bq' x ones ; same for K^T. V[m, d] chunks.
            qT_sb = projpool.tile([D, N], bf16, tag="qT")
            kT_sb = projpool.tile([D, N], bf16, tag="kT")
            v_sb = projpool.tile([128, N], bf16, tag="v")  # chunk m at free 128m

            for c in range(N_NCH):
                sl = slice(c * NCH, (c + 1) * NCH)
                pq = pspool.tile([D, NCH], f32, tag="sc")
                nc.tensor.matmul(pq[:], wq_sb[:], xT_sb[:, sl],
                                 start=True, stop=False)
                nc.tensor.matmul(pq[:], bq_sb[:], ones_sb[:],
                                 start=False, stop=True)
                nc.vector.tensor_copy(qT_sb[:, sl], pq[:])

                pk = pspool.tile([D, NCH], f32, tag="sc")
                nc.tensor.matmul(pk[:], wk_sb[:], condT_sb[:, sl],
                                 start=True, stop=False)
                nc.tensor.matmul(pk[:], bk_sb[:], ones_sb[:],
                                 start=False, stop=True)
                nc.vector.tensor_copy(kT_sb[:, sl], pk[:])

            for t in range(N_MT):
                sl = slice(t * 128, (t + 1) * 128)
                pv = pspool.tile([128, D], f32, tag="sc")
                nc.tensor.matmul(pv[:], condT_sb[:, sl], wv_sb[:],
                                 start=True, stop=False)
                nc.tensor.matmul(pv[:], ones_sb[:, 0:128], bv_sb[:],
                                 start=False, stop=True)  # row0-padded rank-1
                nc.vector.tensor_copy(v_sb[:, sl], pv[:])

            # small ACT-written source tile for the ACT gates below
            actsrc = cpool.tile([1, 8], bf16, tag="actsrc")
            nc.scalar.copy(actsrc[0:1, 0:1], qT_sb[0:1, 0:1])

            # ---- main loop ----
            prev_tanh = []
            prev2_tanh = []
            prev_av = []
            ot_copies = []
            for rep in range(KREP):
              for ncg in range(N_NCH):
                nsl = slice(ncg * NCH, (ncg + 1) * NCH)
                av = avpool.tile([D, NCH], f32, tag="av")
                if len(ot_copies) >= 2:
                    # PE gate: absorbs the av-slot release (DVE oT copy two
                    # ncg back) so the first AV matmul carries only its ACT
                    # wait. Full-height [128, 1] load (see note above).
                    gpe = nc.tensor.ldweights(v_sb[:, 0:1])
                    add_dep_helper(gpe.ins, ot_copies[-2].ins,
                                   reason="av slot release")
                for mt in range(N_MT):
                    mk = mk_tiles[mt]
                    # pair-granular phases: heads (2p, 2p+1) share one
                    # 2-bank PSUM tile; inject/scores/tanh/AV per pair so
                    # the 3 sc slots give 3 pair-steps of PE lookahead
                    # instead of 1.5 whole-group steps.
                    for p in range(2):
                        # ACT gate: absorbs the th-slot WAW (tanh a few
                        # pairs back, same-engine completion wait) so the
                        # real tanh carries only its PE wait. Walrus
                        # rejects >1 sync wait per Activation.
                        gate = None
                        if prev2_tanh:
                            gact = gsbpool.tile(
                                [1, 8], bf16, tag="gact",
                                name=f"gact_{rep}_{ncg}_{mt}_{p}",
                                bufs=132 * KREP)
                            gate = nc.scalar.copy(gact[0:1, 0:1],
                                                  actsrc[0:1, 0:1])
                            for t_ in prev2_tanh:
                                add_dep_helper(gate.ins, t_.ins,
                                               reason="th waw")
                        sc = pspool.tile([128, 2 * NCH], f32, tag="sc",
                                         name=f"sc{p}")
                        # inject mask^T into each head's bank with a
                        # FULL-ARRAY identity matmul (tiled injects at other
                        # row groups racing the tiled score accumulation on
                        # the same PSUM addresses hard-fault the PE).
                        for q in range(2):
                            nc.tensor.matmul(
                                sc[:, q * NCH:(q + 1) * NCH], eyef_sb[:],
                                mk[:, nsl],
                                start=True, stop=False,
                                skip_group_check=True,
                            )
                        # packed per-head scores accumulate on top
                        for q in range(2):
                            h = 2 * p + q
                            hs = slice(32 * h, 32 * (h + 1))
                            for j in range(4):
                                qs = slice(32 * j, 32 * (j + 1))
                                moff = mt * 128 + 32 * j
                                nc.tensor.matmul(
                                    sc[qs, q * NCH:(q + 1) * NCH],
                                    kT_sb[hs, moff:moff + 32],
                                    qT_sb[hs, nsl],
                                    start=False, stop=(j == 3),
                                    tile_position=(32 * h, 32 * j),
                                    skip_group_check=True,
                                )
                        th = thpool.tile([128, 2 * NCH], bf16, tag="th",
                                         name=f"th{p}")
                        prev2_tanh = prev_tanh
                        prev_tanh = []
                        act = nc.scalar.activation(th[:], sc[:], TANH)
                        prev_tanh.append(act)
                        # head-mean via linearity: av += V'[mt]^T @ tanh_h^T
                        for q in range(2):
                            h = 2 * p + q
                            mm = nc.tensor.matmul(
                                av[:], v_sb[:, mt * 128:(mt + 1) * 128],
                                th[:, q * NCH:(q + 1) * NCH],
                                start=(mt == 0 and h == 0),
                                stop=(mt == N_MT - 1 and h == H - 1),
                            )
                        prev_av = [mm]
                oT = opool.tile([D, NCH], f32, tag="oT", bufs=4 * KREP)
                cp = nc.vector.tensor_copy(oT[:], av[:])
                ot_copies.append(cp)
                if rep == KREP - 1:
                    od = nc.gpsimd.dma_start(out=outT[ncg][:], in_=oT[:])
                    tail_insts.extend([cp, od])

            tail_insts.extend(prev_tanh)
            tail_insts.extend(prev_av)
            for ti in tail_insts:
                nz = nc.sync.nop(nofuse=True, hint="predrain")
                add_dep_helper(nz.ins, ti.ins, reason="predrain absorb")

    return nc


def get_nc():
    if "nc" not in _NC_CACHE:
        _NC_CACHE["nc"] = _build_nc()
    return _NC_CACHE["nc"]


def _prep_in_maps(x, cond, attention_mask, Wq, bq, Wk, bk, Wv, bv):
    import ml_dtypes

    bf16 = ml_dtypes.bfloat16
    s = 1.0 / math.sqrt(128.0)

    Wq_s = (np.asarray(Wq, np.float32) * s).astype(bf16)
    Wk_b = np.asarray(Wk, np.float32).astype(bf16)
    Wv4 = (np.asarray(Wv, np.float32) * 0.25).astype(bf16)

    def _row0(vec):
        m = np.zeros((D, D), np.float32)
        m[0, :] = vec
        return m.astype(bf16)

    bq_s = _row0(np.asarray(bq, np.float32) * s)
    bk_b = _row0(np.asarray(bk, np.float32))
    bv4 = _row0(np.asarray(bv, np.float32) * 0.25)
    onesm = np.zeros((D, NCH), np.float32)
    onesm[0, :] = 1.0
    onesm = onesm.astype(bf16)
    eyef = np.eye(D, dtype=np.float32).astype(bf16)

    x = np.asarray(x, np.float32)
    cond = np.asarray(cond, np.float32)
    attention_mask = np.asarray(attention_mask, np.float32)

    in_maps = []
    for i in range(B):
        in_maps.append({
            "xT": np.ascontiguousarray(x[i].T).astype(bf16),
            "condT": np.ascontiguousarray(cond[i].T).astype(bf16),
            "maskT": np.ascontiguousarray(attention_mask[i].T).astype(bf16),
            "Wq": Wq_s, "Wk": Wk_b, "Wv4": Wv4,
            "bq": bq_s, "bk": bk_b, "bv4": bv4,
            "onesm": onesm, "eyef": eyef,
        })
    return in_maps


def run(x, cond, flags, attention_mask, Wq, bq, Wk, bk, Wv, bv,
        trace=False, tmpdir=None):
    """Returns (out [B,N,D] float32, exec_time_ns or None)."""
    from concourse.bass_utils import run_bass_kernel_spmd

    nc = get_nc()
    in_maps = _prep_in_maps(x, cond, attention_mask, Wq, bq, Wk, bk, Wv, bv)
    res = run_bass_kernel_spmd(
        nc, in_maps, core_ids=list(range(B)), trace=trace, tmpdir=tmpdir,
    )
    out = np.stack(
        [np.concatenate([np.asarray(r[f"outT{i}"], np.float32)
                         for i in range(N_NCH)], axis=1).T
         for r in res.results], axis=0
    )
    return out, res.exec_time_ns


def kernel(**inputs):
    out, _ = run(**inputs)
    return out


# revision 16
# speedup vs baseline: 1.8076x; 1.8076x over previous
"""Trainium2 Bass kernel for nn_Attention_65747359367242.

Per-batch tanh-attention with head-mean:
  Q = x@Wq+bq, K = cond@Wk+bk, V = cond@Wv+bv   (4 heads of 32 dims)
  S_h = Q_h K_h^T / sqrt(128)
  A   = mean_h tanh(mask + S_h)
  out = A @ V

Sharding: pure data-parallel, batch b -> core b (B=8, 8 cores). No collectives.

Device strategy per core (transposed orientation: scores S^T[m, n]):
  - host feeds x^T, cond^T, mask^T (bf16) + prescaled weights
  - Q^T/K^T/V computed on device via small matmuls (biases added as rank-1
    matmuls accumulating into the same PSUM)
  - main loop over (ncg: 4 n-chunks of 512) x (mt: 16 m-tiles of 128):
      * mask^T tile injected into 4 PSUM half-banks (one per head) via
        full-array identity matmuls (start=True clears, sets has_written)
      * 16 score matmuls (4 heads x 4 m-subtiles, K=32) packed at the 16
        32x32 tile positions accumulate S_h^T on top -> PSUM = mask + S_h
      * ScalarE tanh PSUM -> SBUF bf16, one per head-PAIR (FD=1024)
      * head-mean folded into AV by linearity: av[d, n] accumulates one
        matmul per head, moving operand = tanh slice (V' = Wv/4 prescaled)
  - out^T streamed to DRAM; host transposes back.

The ScalarE tanh stream (~128 us/core) is the theoretical bottleneck;
everything else (PE ~75 us, DVE ~25 us, DMA ~25 us) pipelines underneath.
"""

import math
import os
import sys

import numpy as np

sys.path.insert(0, "/opt/trn_rl_repo")

KREP = int(os.environ.get("KREP", "1"))  # on-device repeats of main loop

B, N, D = 8, 2048, 128
H, DH = 4, 32
NCH = 512            # n-chunk (free dim of score tiles / psum bank)
N_NCH = N // NCH     # 4
N_MT = N // 128      # 16 m-tiles

_NC_CACHE = {}


def _build_nc():
    from concourse import bass, tile
    from concourse.tile import add_dep_helper

    mybir = sys.modules["concourse.mybir"]
    f32 = mybir.dt.float32
    bf16 = mybir.dt.bfloat16
    TANH = mybir.ActivationFunctionType.Tanh

    nc = bass.Bass()

    xT = nc.declare_dram_parameter("xT", [D, N], bf16, isOutput=False)
    condT = nc.declare_dram_parameter("condT", [D, N], bf16, isOutput=False)
    maskT = nc.declare_dram_parameter("maskT", [N, N], bf16, isOutput=False)
    Wq = nc.declare_dram_parameter("Wq", [D, D], bf16, isOutput=False)
    Wk = nc.declare_dram_parameter("Wk", [D, D], bf16, isOutput=False)
    Wv4 = nc.declare_dram_parameter("Wv4", [D, D], bf16, isOutput=False)
    bq = nc.declare_dram_parameter("bq", [D, D], bf16, isOutput=False)
    bk = nc.declare_dram_parameter("bk", [D, D], bf16, isOutput=False)
    bv4 = nc.declare_dram_parameter("bv4", [D, D], bf16, isOutput=False)
    onesm = nc.declare_dram_parameter("onesm", [D, NCH], bf16, isOutput=False)
    eyef = nc.declare_dram_parameter("eyef", [D, D], bf16, isOutput=False)
    outT = [nc.declare_dram_parameter(f"outT{i}", [D, NCH], f32,
                                      isOutput=True) for i in range(N_NCH)]

    with tile.TileContext(nc) as tc:
        with (
            tc.tile_pool(name="const", bufs=1) as cpool,
            tc.tile_pool(name="proj", bufs=1) as projpool,
            tc.tile_pool(name="mask", bufs=16) as mpool,
            tc.tile_pool(name="th", bufs=6) as thpool,
            tc.tile_pool(name="osb", bufs=4 * KREP) as opool,
            tc.tile_pool(name="ps", bufs=3, space="PSUM") as pspool,
            tc.tile_pool(name="av", bufs=2, space="PSUM") as avpool,
            tc.tile_pool(name="gsb", bufs=66 * KREP) as gsbpool,
        ):
            # ---- load constants / inputs ----
            wq_sb = cpool.tile([D, D], bf16, tag="wq")
            wk_sb = cpool.tile([D, D], bf16, tag="wk")
            wv_sb = cpool.tile([D, D], bf16, tag="wv")
            bq_sb = cpool.tile([D, D], bf16, tag="bq")
            bk_sb = cpool.tile([D, D], bf16, tag="bk")
            bv_sb = cpool.tile([D, D], bf16, tag="bv")
            ones_sb = cpool.tile([D, NCH], bf16, tag="ones")
            eyef_sb = cpool.tile([D, D], bf16, tag="eyef")
            xT_sb = cpool.tile([D, N], bf16, tag="xT")
            condT_sb = cpool.tile([D, N], bf16, tag="condT")

            # ldweights gates absorb DMA waits on the PE side (the Matmult
            # HW struct fits only one sync wait). They must be FULL-HEIGHT
            # [128, 1] loads: partial-height standalone ldweights before
            # tile_position matmuls hard-fault the PE
            # (NRT_EXEC_UNIT_UNRECOVERABLE).
            for sb_t, dr_t in [(wq_sb, Wq), (wk_sb, Wk), (wv_sb, Wv4),
                               (eyef_sb, eyef), (xT_sb, xT),
                               (condT_sb, condT)]:
                nc.sync.dma_start(out=sb_t[:], in_=dr_t[:])
                nc.tensor.ldweights(sb_t[:, 0:1])
            for sb_t, dr_t in [(bq_sb, bq), (bk_sb, bk), (bv_sb, bv4),
                               (ones_sb, onesm)]:
                nc.sync.dma_start(out=sb_t[:], in_=dr_t[:])
                nc.tensor.ldweights(sb_t[:, 0:1])

            # ---- mask prefetch: 16 full-row DMAs [128, 2048] (4 KB/line) ----
            mk_tiles = []
            tail_insts = []
            for mt in range(N_MT):
                mk = mpool.tile([128, N], bf16, tag="mk",
                                name=f"mk_{mt}", bufs=16)
                dmi = nc.sync.dma_start(
                    out=mk[:], in_=maskT[mt * 128:(mt + 1) * 128, :])
                # gate: absorbs the DMA wait so inject matmuls carry only
                # their slot-release wait
                nc.tensor.ldweights(mk[:, 0:1])
                mk_tiles.append(mk)
                if mt >= N_MT - 8:
                    # predrain nops (below) absorb these lanes' final DMAHW
                    # ticks so the kernel-tail drain fits its wait slots
                    tail_insts.append(dmi)

            # ---- projections ----
            # Q^T[d, n] = Wq'^T x^T + bq' x ones ; same for K^T. V[m, d] chunks.
            qT_sb = projpool.tile([D, N], bf16, tag="qT")
            kT_sb = projpool.tile([D, N], bf16, tag="kT")
            v_sb = projpool.tile([128, N], bf16, tag="v")  # chunk m at free 128m

            for c in range(N_NCH):
                sl = slice(c * NCH, (c + 1) * NCH)
                pq = pspool.tile([D, NCH], f32, tag="sc")
                nc.tensor.matmul(pq[:], wq_sb[:], xT_sb[:, sl],
                                 start=True, stop=False)
                nc.tensor.matmul(pq[:], bq_sb[:], ones_sb[:],
                                 start=False, stop=True)
                nc.vector.tensor_copy(qT_sb[:, sl], pq[:])

                pk = pspool.tile([D, NCH], f32, tag="sc")
                nc.tensor.matmul(pk[:], wk_sb[:], condT_sb[:, sl],
                                 start=True, stop=False)
                nc.tensor.matmul(pk[:], bk_sb[:], ones_sb[:],
                                 start=False, stop=True)
                nc.vector.tensor_copy(kT_sb[:, sl], pk[:])

            for t in range(N_MT):
                sl = slice(t * 128, (t + 1) * 128)
                pv = pspool.tile([128, D], f32, tag="sc")
                nc.tensor.matmul(pv[:], condT_sb[:, sl], wv_sb[:],
                                 start=True, stop=False)
                nc.tensor.matmul(pv[:], ones_sb[:, 0:128], bv_sb[:],
                                 start=False, stop=True)  # row0-padded rank-1
                nc.vector.tensor_copy(v_sb[:, sl], pv[:])

            # small ACT-written source tile for the ACT gates below
            actsrc = cpool.tile([1, 8], bf16, tag="actsrc")
            nc.scalar.copy(actsrc[0:1, 0:1], qT_sb[0:1, 0:1])

            # ---- main loop ----
            # Software-pipelined: group g's inject+scores (PE) and tanhs
            # (ACT) are emitted first; group g-1's AV matmuls are emitted
            # AFTER group g's scores so they never head-of-line-block the
            # PE queue while waiting on tanh(g-1).
            tanh_hist = []   # per-group tanh instruction lists
            prev_av = []
            ot_copies = []
            pending = None   # (th_tile, mt, av_tile, ncg, rep) awaiting AV
            prev_score = None  # last score matmul of the previous group

            def emit_av(pend, is_last):
                th, mt, av_t, ncg_, rep_ = pend
                last = None
                for h in range(H):
                    p, q = divmod(h, 2)
                    last = nc.tensor.matmul(
                        av_t[:], v_sb[:, mt * 128:(mt + 1) * 128],
                        th[p][:, q * NCH:(q + 1) * NCH],
                        start=(mt == 0 and h == 0),
                        stop=(mt == N_MT - 1 and h == H - 1),
                    )
                if mt == N_MT - 1:
                    oT = opool.tile([D, NCH], f32, tag="oT",
                                    name=f"oT_{rep_}_{ncg_}", bufs=4 * KREP)
                    cp = nc.vector.tensor_copy(oT[:], av_t[:])
                    ot_copies.append(cp)
                    if rep_ == KREP - 1:
                        od = nc.gpsimd.dma_start(out=outT[ncg_][:],
                                                 in_=oT[:])
                        tail_insts.extend([cp, od])
                return last

            for rep in range(KREP):
              for ncg in range(N_NCH):
                nsl = slice(ncg * NCH, (ncg + 1) * NCH)
                av = avpool.tile([D, NCH], f32, tag="av")
                if rep * N_NCH + ncg >= 2:
                    # PE gate: absorbs the av-slot release (DVE oT copy of
                    # the ncg two back; with the one-group AV pipeline its
                    # copy is the latest in ot_copies) so the first AV
                    # matmul carries only its ACT wait. Full-height
                    # [128, 1] load (see note above).
                    gpe = nc.tensor.ldweights(v_sb[:, 0:1])
                    add_dep_helper(gpe.ins, ot_copies[-1].ins,
                                   reason="av slot release")
                for mt in range(N_MT):
                    mk = mk_tiles[mt]
                    # ACT gate: absorbs the th-slot WAW (same-engine
                    # completion wait, tanh three groups back at bufs=6) so
                    # the real tanhs carry only their PE wait. Walrus
                    # rejects >1 sync wait per Activation. Dep is TWO
                    # groups back: late enough to cover the slot reuse,
                    # old enough to be complete on arrival (a one-group-
                    # back dep stalls ACT ~2us/group).
                    gate = None
                    if len(tanh_hist) >= 2:
                        gact = gsbpool.tile([1, 8], bf16, tag="gact",
                                            name=f"gact_{rep}_{ncg}_{mt}",
                                            bufs=66 * KREP)
                        gate = nc.scalar.copy(gact[0:1, 0:1],
                                              actsrc[0:1, 0:1])
                        for t_ in tanh_hist[-2]:
                            add_dep_helper(gate.ins, t_.ins,
                                           reason="th waw")
                    if tanh_hist:
                        # PE gate: absorbs the ACT sc-slot-release wait
                        # (tanh A of the previous group frees the slot the
                        # second inject needs; it also covers the older
                        # tanh B release) so the injects carry only their
                        # PE WAW wait (packed scores of the previous group
                        # still in flight when a full-array inject reuses
                        # their PSUM slot).
                        gw = nc.tensor.ldweights(eyef_sb[:, 0:1])
                        add_dep_helper(gw.ins, tanh_hist[-1][0].ins,
                                       reason="sc slot release")
                    # paired-head PSUM tiles: heads (0,1) share tile A
                    # (2 banks), heads (2,3) share tile B -> one tanh per
                    # tile at FD=1024 instead of two at FD=512.
                    sc = [pspool.tile([128, 2 * NCH], f32, tag="sc",
                                      name=f"sc{p}") for p in range(2)]
                    # inject mask^T into each head's bank with a FULL-ARRAY
                    # identity matmul (tiled injects at other row groups
                    # racing the tiled score accumulation on the same PSUM
                    # addresses hard-fault the PE).
                    for h in range(H):
                        off = NCH * (h % 2)
                        nc.tensor.matmul(
                            sc[h // 2][:, off:off + NCH], eyef_sb[:],
                            mk[:, nsl],
                            start=True, stop=False,
                            skip_group_check=True,
                        )
                    # packed per-head scores accumulate on top
                    for j in range(4):
                        qs = slice(32 * j, 32 * (j + 1))
                        moff = mt * 128 + 32 * j
                        for h in range(H):
                            hs = slice(32 * h, 32 * (h + 1))
                            off = NCH * (h % 2)
                            prev_score = nc.tensor.matmul(
                                sc[h // 2][qs, off:off + NCH],
                                kT_sb[hs, moff:moff + 32],
                                qT_sb[hs, nsl],
                                start=False, stop=(j == 3),
                                tile_position=(32 * h, 32 * j),
                                skip_group_check=True,
                            )
                    th = [thpool.tile([128, 2 * NCH], bf16, tag="th",
                                      name=f"th{p}") for p in range(2)]
                    cur_tanh = []
                    for p in range(2):
                        act = nc.scalar.activation(th[p][:], sc[p][:], TANH)
                        cur_tanh.append(act)
                    tanh_hist.append(cur_tanh)
                    # AV matmuls for the PREVIOUS group, one group late
                    if pending is not None:
                        prev_av = [emit_av(pending, False)]
                    pending = (th, mt, av, ncg, rep)
            prev_av = [emit_av(pending, True)]

            tail_insts.extend(tanh_hist[-1])
            tail_insts.extend(prev_av)
            for ti in tail_insts:
                nz = nc.sync.nop(nofuse=True, hint="predrain")
                add_dep_helper(nz.ins, ti.ins, reason="predrain absorb")

    return nc


def get_nc():
    if "nc" not in _NC_CACHE:
        _NC_CACHE["nc"] = _build_nc()
    return _NC_CACHE["nc"]


def _prep_in_maps(x, cond, attention_mask, Wq, bq, Wk, bk, Wv, bv):
    import ml_dtypes

    bf16 = ml_dtypes.bfloat16
    s = 1.0 / math.sqrt(128.0)

    Wq_s = (np.asarray(Wq, np.float32) * s).astype(bf16)
    Wk_b = np.asarray(Wk, np.float32).astype(bf16)
    Wv4 = (np.asarray(Wv, np.float32) * 0.25).astype(bf16)

    def _row0(vec):
        m = np.zeros((D, D), np.float32)
        m[0, :] = vec
        return m.astype(bf16)

    bq_s = _row0(np.asarray(bq, np.float32) * s)
    bk_b = _row0(np.asarray(bk, np.float32))
    bv4 = _row0(np.asarray(bv, np.float32) * 0.25)
    onesm = np.zeros((D, NCH), np.float32)
    onesm[0, :] = 1.0
    onesm = onesm.astype(bf16)
    eyef = np.eye(D, dtype=np.float32).astype(bf16)

    x = np.asarray(x, np.float32)
    cond = np.asarray(cond, np.float32)
    attention_mask = np.asarray(attention_mask, np.float32)

    in_maps = []
    for i in range(B):
        in_maps.append({
            "xT": np.ascontiguousarray(x[i].T).astype(bf16),
            "condT": np.ascontiguousarray(cond[i].T).astype(bf16),
            "maskT": np.ascontiguousarray(attention_mask[i].T).astype(bf16),
            "Wq": Wq_s, "Wk": Wk_b, "Wv4": Wv4,
            "bq": bq_s, "bk": bk_b, "bv4": bv4,
            "onesm": onesm, "eyef": eyef,
        })
    return in_maps


def run(x, cond, flags, attention_mask, Wq, bq, Wk, bk, Wv, bv,
        trace=False, tmpdir=None):
    """Returns (out [B,N,D] float32, exec_time_ns or None)."""
    from concourse.bass_utils import run_bass_kernel_spmd

    nc = get_nc()
    in_maps = _prep_in_maps(x, cond, attention_mask, Wq, bq, Wk, bk, Wv, bv)
    res = run_bass_kernel_spmd(
        nc, in_maps, core_ids=list(range(B)), trace=trace, tmpdir=tmpdir,
    )
    out = np.stack(
        [np.concatenate([np.asarray(r[f"outT{i}"], np.float32)
                         for i in range(N_NCH)], axis=1).T
         for r in res.results], axis=0
    )
    return out, res.exec_time_ns


def kernel(**inputs):
    out, _ = run(**inputs)
    return out


# revision 24
# speedup vs baseline: 2.0361x; 1.1264x over previous
"""Trainium2 Bass kernel for nn_Attention_65747359367242.

Per-batch tanh-attention with head-mean:
  Q = x@Wq+bq, K = cond@Wk+bk, V = cond@Wv+bv   (4 heads of 32 dims)
  S_h = Q_h K_h^T / sqrt(128)
  A   = mean_h tanh(mask + S_h)
  out = A @ V

Sharding: pure data-parallel, batch b -> core b (B=8, 8 cores). No collectives.

Device strategy per core (transposed orientation: scores S^T[m, n]):
  - host feeds x^T, cond^T, mask^T (bf16) + prescaled packed weights
  - 4 big mask DMAs ([128, 4, 2048], 2 MB each) issued first, then one
    packed const DMA, then x^T/cond^T
  - Q^T/K^T/V computed on device via small matmuls (biases added as rank-1
    matmuls accumulating into the same PSUM)
  - main loop over (ncg: 4 n-chunks of 512) x (mt: 16 m-tiles of 128),
    with head-PAIR phases inside each group:
      * gate (PE ldweights) absorbing the ACT slot-release wait
      * mask^T inject into the pair's 2 PSUM banks (full-array identity
        matmuls), 8 packed score matmuls (K=32 tile positions) on top
      * ScalarE tanh PSUM -> SBUF bf16 at FD=1024 per pair
      * AV matmuls (head-mean by linearity, V' = Wv/4) emitted ONE GROUP
        LATE so they never head-of-line-block the PE queue
  - out^T streamed to DRAM (HWDGE); host transposes back.

Steady state is ScalarE-bound (~2.4 us per group: 2 tanhs + amortized
gate); PE runs ~1.5 groups ahead, paced by PSUM slot releases.
"""

import math
import os
import sys

import numpy as np

sys.path.insert(0, "/opt/trn_rl_repo")

KREP = int(os.environ.get("KREP", "1"))  # on-device repeats of main loop

B, N, D = 8, 2048, 128
H, DH = 4, 32
NCH = 512            # n-chunk (free dim of score tiles / psum bank)
N_NCH = N // NCH     # 4
N_MT = N // 128      # 16 m-tiles

# packed const layout (columns in cpack)
_CW = {"wq": 0, "wk": 128, "wv": 256, "bq": 384, "bk": 512, "bv": 640,
       "eyef": 768, "ones": 896}
CPACK_COLS = 896 + NCH

_NC_CACHE = {}


def _build_nc():
    from concourse import bass, tile
    from concourse.tile import add_dep_helper

    mybir = sys.modules["concourse.mybir"]
    f32 = mybir.dt.float32
    bf16 = mybir.dt.bfloat16
    TANH = mybir.ActivationFunctionType.Tanh

    nc = bass.Bass()

    xT = nc.declare_dram_parameter("xT", [D, N], bf16, isOutput=False)
    condT = nc.declare_dram_parameter("condT", [D, N], bf16, isOutput=False)
    maskT = nc.declare_dram_parameter("maskT", [N, N], bf16, isOutput=False)
    cpack = nc.declare_dram_parameter("cpack", [D, CPACK_COLS], bf16,
                                      isOutput=False)
    outT = [nc.declare_dram_parameter(f"outT{i}", [D, NCH], f32,
                                      isOutput=True) for i in range(N_NCH)]

    with tile.TileContext(nc) as tc:
        with (
            tc.tile_pool(name="const", bufs=1) as cpool,
            tc.tile_pool(name="proj", bufs=1) as projpool,
            tc.tile_pool(name="mask", bufs=4) as mpool,
            tc.tile_pool(name="th", bufs=6) as thpool,
            tc.tile_pool(name="osb", bufs=4 * KREP) as opool,
            tc.tile_pool(name="ps", bufs=3, space="PSUM") as pspool,
            tc.tile_pool(name="av", bufs=2, space="PSUM") as avpool,
            tc.tile_pool(name="gsb", bufs=40 * KREP) as gsbpool,
        ):
            tail_insts = []

            # ---- mask prefetch first: 4 DMAs of [128, 4, 2048] (4 KB
            # lines, 2 MB each). No PE gates: the first inject per big
            # tile carries the DMA wait directly.
            mk_tiles = []
            mk_dmas = []
            for t in range(4):
                mk = mpool.tile([128, 4, N], bf16, tag="mk",
                                name=f"mk_{t}", bufs=4)
                dmi = nc.sync.dma_start(
                    out=mk[:],
                    in_=maskT[t * 512:(t + 1) * 512, :].rearrange(
                        "(c p) n -> p c n", p=128))
                mk_dmas.append(dmi)
                mk_tiles.append(mk)

            # ---- load consts / inputs ----
            # ldweights gates absorb DMA waits on the PE side (the Matmult
            # HW struct fits only one sync wait). They must be FULL-HEIGHT
            # [128, 1] loads: partial-height standalone ldweights before
            # tile_position matmuls hard-fault the PE
            # (NRT_EXEC_UNIT_UNRECOVERABLE).
            cp_sb = cpool.tile([D, CPACK_COLS], bf16, tag="cpack")
            xT_sb = cpool.tile([D, N], bf16, tag="xT")
            condT_sb = cpool.tile([D, N], bf16, tag="condT")
            # const/input loads go via SWDGE (gpsimd) so the 4 HWDGE mask
            # DMAs keep lanes 0-3 and the 4 output DMAs land on fresh
            # lanes 4-7 (lane reuse would add a second, non-absorbable
            # FIFO wait to the output DMA_DIRECT2D).
            in_dmas = list(mk_dmas)
            for sb_t, dr_t in [(cp_sb, cpack), (xT_sb, xT),
                               (condT_sb, condT)]:
                dmi = nc.gpsimd.dma_start(out=sb_t[:], in_=dr_t[:])
                nc.tensor.ldweights(sb_t[:, 0:1])
                in_dmas.append(dmi)

            wq_sb = cp_sb[:, _CW["wq"]:_CW["wq"] + D]
            wk_sb = cp_sb[:, _CW["wk"]:_CW["wk"] + D]
            wv_sb = cp_sb[:, _CW["wv"]:_CW["wv"] + D]
            bq_sb = cp_sb[:, _CW["bq"]:_CW["bq"] + D]
            bk_sb = cp_sb[:, _CW["bk"]:_CW["bk"] + D]
            bv_sb = cp_sb[:, _CW["bv"]:_CW["bv"] + D]
            eyef_sb = cp_sb[:, _CW["eyef"]:_CW["eyef"] + D]
            ones_sb = cp_sb[:, _CW["ones"]:_CW["ones"] + NCH]

            # ---- projections ----
            # Q^T[d, n] = Wq'^T x^T + bq' x ones ; same for K^T. V[m, d] chunks.
            qT_sb = projpool.tile([D, N], bf16, tag="qT")
            kT_sb = projpool.tile([D, N], bf16, tag="kT")
            v_sb = projpool.tile([128, N], bf16, tag="v")  # chunk m at free 128m

            for c in range(N_NCH):
                sl = slice(c * NCH, (c + 1) * NCH)
                pq = pspool.tile([D, NCH], f32, tag="sc")
                nc.tensor.matmul(pq[:], wq_sb, xT_sb[:, sl],
                                 start=True, stop=False)
                nc.tensor.matmul(pq[:], bq_sb, ones_sb,
                                 start=False, stop=True)
                nc.vector.tensor_copy(qT_sb[:, sl], pq[:])

                pk = pspool.tile([D, NCH], f32, tag="sc")
                nc.tensor.matmul(pk[:], wk_sb, condT_sb[:, sl],
                                 start=True, stop=False)
                nc.tensor.matmul(pk[:], bk_sb, ones_sb,
                                 start=False, stop=True)
                nc.vector.tensor_copy(kT_sb[:, sl], pk[:])

            last_proj_cp = None
            for t in range(N_MT):
                sl = slice(t * 128, (t + 1) * 128)
                pv = pspool.tile([128, D], f32, tag="sc")
                nc.tensor.matmul(pv[:], condT_sb[:, sl], wv_sb,
                                 start=True, stop=False)
                nc.tensor.matmul(pv[:], ones_sb[:, 0:128], bv_sb,
                                 start=False, stop=True)  # row0-padded rank-1
                last_proj_cp = nc.vector.tensor_copy(v_sb[:, sl], pv[:])

            # small ACT-written source tile for the ACT gates below
            actsrc = cpool.tile([1, 8], bf16, tag="actsrc")
            nc.scalar.copy(actsrc[0:1, 0:1], qT_sb[0:1, 0:1])

            # PE gate: absorbs the DVE release of the proj PSUM slots the
            # first main-loop sc tiles reuse.
            gpre = nc.tensor.ldweights(eyef_sb[:, 0:1])
            add_dep_helper(gpre.ins, last_proj_cp.ins, reason="proj release")

            # Early sync nops: advance the sync engine's observed DMAHW
            # lane ticks past the input DMAs, so the output DMAs and the
            # kernel-tail drain don't carry those waits themselves.
            for dmi in in_dmas:
                nz = nc.sync.nop(nofuse=True, hint="indma")
                add_dep_helper(nz.ins, dmi.ins, reason="in dma absorb")

            # ---- main loop ----
            # Group g's inject+scores (PE) and tanhs (ACT) are emitted
            # first; group g-1's AV matmuls are emitted AFTER so they never
            # head-of-line-block the PE queue while waiting on tanh(g-1).
            tanh_hist = []   # per-group [tanhA, tanhB]
            prev_av = []
            ot_copies = []
            pending = None   # (th_pair, mt, av_tile, ncg, rep) awaiting AV
            gidx = 0

            def emit_av(pend):
                th, mt, av_t, ncg_, rep_ = pend
                last = None
                for h in range(H):
                    p, q = divmod(h, 2)
                    last = nc.tensor.matmul(
                        av_t[:], v_sb[:, mt * 128:(mt + 1) * 128],
                        th[p][:, q * NCH:(q + 1) * NCH],
                        start=(mt == 0 and h == 0),
                        stop=(mt == N_MT - 1 and h == H - 1),
                    )
                if mt == N_MT - 1:
                    oT = opool.tile([D, NCH], f32, tag="oT",
                                    name=f"oT_{rep_}_{ncg_}", bufs=4 * KREP)
                    cp = nc.vector.tensor_copy(oT[:], av_t[:])
                    ot_copies.append(cp)
                    if rep_ == KREP - 1:
                        od = nc.sync.dma_start(out=outT[ncg_][:], in_=oT[:])
                        tail_insts.extend([cp, od])
                return last

            for rep in range(KREP):
              for ncg in range(N_NCH):
                nsl = slice(ncg * NCH, (ncg + 1) * NCH)
                av = avpool.tile([D, NCH], f32, tag="av")
                if rep * N_NCH + ncg >= 2:
                    # PE gate: absorbs the av-slot release (DVE oT copy of
                    # the ncg two back) so the first AV matmul carries only
                    # its ACT wait.
                    gpe = nc.tensor.ldweights(eyef_sb[:, 0:1])
                    add_dep_helper(gpe.ins, ot_copies[-1].ins,
                                   reason="av slot release")
                for mt in range(N_MT):
                    mkt = mk_tiles[mt // 4]
                    mk_sl = mkt[:, mt % 4, nsl]
                    if rep == 0 and ncg == 0 and mt % 4 == 0:
                        # PE gate: absorbs this big mask tile's DMA wait
                        # just before its first use (placing all mask gates
                        # up front would head-of-line-block the PE queue
                        # until the whole 8 MB mask stream lands).
                        gm = nc.tensor.ldweights(mkt[:, 0, 0:1])
                        add_dep_helper(gm.ins, mk_dmas[mt // 4].ins,
                                       reason="mask dma")
                    # ACT gate every 2nd group: absorbs the th-slot WAW
                    # (same-engine completion wait; tanh(g) reuses the slot
                    # of tanh(g-3) at bufs=6, and a dep on tanh(g-2) covers
                    # groups g and g+1). Walrus rejects >1 sync wait per
                    # Activation.
                    if gidx >= 3 and gidx % 2 == 1:
                        gact = gsbpool.tile([1, 8], bf16, tag="gact",
                                            name=f"gact_{gidx}",
                                            bufs=40 * KREP)
                        ga = nc.scalar.copy(gact[0:1, 0:1],
                                            actsrc[0:1, 0:1])
                        for t_ in tanh_hist[-2]:
                            add_dep_helper(ga.ins, t_.ins, reason="th waw")
                    cur_tanh = []
                    th = []
                    sc_pair = []
                    for p in range(2):
                        # PE gate: absorbs the ACT sc-slot-release wait
                        # (tanh of the group that last used this slot) so
                        # the injects carry only their PE WAW wait (packed
                        # scores of an older group may still be in flight
                        # when a full-array inject reuses their PSUM slot).
                        rel = None
                        if p == 0 and gidx >= 2:
                            rel = tanh_hist[-2][1]   # tanhB(g-2)
                        elif p == 1 and gidx >= 1:
                            rel = tanh_hist[-1][0]   # tanhA(g-1)
                        if rel is not None:
                            gw = nc.tensor.ldweights(eyef_sb[:, 0:1])
                            add_dep_helper(gw.ins, rel.ins,
                                           reason="sc slot release")
                        # paired-head PSUM tile: heads (2p, 2p+1) share a
                        # 2-bank tile -> one tanh at FD=1024.
                        sc = pspool.tile([128, 2 * NCH], f32, tag="sc",
                                         name=f"sc{p}")
                        sc_pair.append(sc)
                        # inject mask^T into each head's bank with a
                        # FULL-ARRAY identity matmul (tiled injects at
                        # other row groups racing the tiled score
                        # accumulation on the same PSUM addresses
                        # hard-fault the PE).
                        for q in range(2):
                            nc.tensor.matmul(
                                sc[:, q * NCH:(q + 1) * NCH], eyef_sb,
                                mk_sl,
                                start=True, stop=False,
                                skip_group_check=True,
                            )
                        # packed per-head scores accumulate on top
                        for q in range(2):
                            h = 2 * p + q
                            hs = slice(32 * h, 32 * (h + 1))
                            for j in range(4):
                                qs = slice(32 * j, 32 * (j + 1))
                                moff = mt * 128 + 32 * j
                                nc.tensor.matmul(
                                    sc[qs, q * NCH:(q + 1) * NCH],
                                    kT_sb[hs, moff:moff + 32],
                                    qT_sb[hs, nsl],
                                    start=False, stop=(j == 3),
                                    tile_position=(32 * h, 32 * j),
                                    skip_group_check=True,
                                )
                        tht = thpool.tile([128, 2 * NCH], bf16, tag="th",
                                          name=f"th{p}")
                        th.append(tht)
                        act = nc.scalar.activation(tht[:], sc[:], TANH)
                        cur_tanh.append(act)
                    tanh_hist.append(cur_tanh)
                    # AV matmuls for the PREVIOUS group, one group late
                    if pending is not None:
                        prev_av = [emit_av(pending)]
                    pending = (th, mt, av, ncg, rep)
                    gidx += 1
            prev_av = [emit_av(pending)]

            tail_insts.extend(tanh_hist[-1])
            tail_insts.extend(prev_av)
            for ti in tail_insts:
                nz = nc.sync.nop(nofuse=True, hint="predrain")
                add_dep_helper(nz.ins, ti.ins, reason="predrain absorb")

    return nc


def get_nc():
    if "nc" not in _NC_CACHE:
        _NC_CACHE["nc"] = _build_nc()
    return _NC_CACHE["nc"]


def _prep_in_maps(x, cond, attention_mask, Wq, bq, Wk, bk, Wv, bv):
    import ml_dtypes

    bf16 = ml_dtypes.bfloat16
    s = 1.0 / math.sqrt(128.0)

    cpack = np.zeros((D, CPACK_COLS), np.float32)
    cpack[:, _CW["wq"]:_CW["wq"] + D] = np.asarray(Wq, np.float32) * s
    cpack[:, _CW["wk"]:_CW["wk"] + D] = np.asarray(Wk, np.float32)
    cpack[:, _CW["wv"]:_CW["wv"] + D] = np.asarray(Wv, np.float32) * 0.25
    cpack[0, _CW["bq"]:_CW["bq"] + D] = np.asarray(bq, np.float32) * s
    cpack[0, _CW["bk"]:_CW["bk"] + D] = np.asarray(bk, np.float32)
    cpack[0, _CW["bv"]:_CW["bv"] + D] = np.asarray(bv, np.float32) * 0.25
    cpack[:, _CW["eyef"]:_CW["eyef"] + D] = np.eye(D, dtype=np.float32)
    cpack[0, _CW["ones"]:_CW["ones"] + NCH] = 1.0
    cpack = cpack.astype(bf16)

    x = np.asarray(x, np.float32)
    cond = np.asarray(cond, np.float32)
    attention_mask = np.asarray(attention_mask, np.float32)

    in_maps = []
    for i in range(B):
        in_maps.append({
            "xT": np.ascontiguousarray(x[i].T).astype(bf16),
            "condT": np.ascontiguousarray(cond[i].T).astype(bf16),
            "maskT": np.ascontiguousarray(attention_mask[i].T).astype(bf16),
            "cpack": cpack,
        })
    return in_maps


def run(x, cond, flags, attention_mask, Wq, bq, Wk, bk, Wv, bv,
        trace=False, tmpdir=None):
    """Returns (out [B,N,D] float32, exec_time_ns or None)."""
    from concourse.bass_utils import run_bass_kernel_spmd

    nc = get_nc()
    in_maps = _prep_in_maps(x, cond, attention_mask, Wq, bq, Wk, bk, Wv, bv)
    res = run_bass_kernel_spmd(
        nc, in_maps, core_ids=list(range(B)), trace=trace, tmpdir=tmpdir,
    )
    out = np.stack(
        [np.concatenate([np.asarray(r[f"outT{i}"], np.float32)
                         for i in range(N_NCH)], axis=1).T
         for r in res.results], axis=0
    )
    return out, res.exec_time_ns


def kernel(**inputs):
    out, _ = run(**inputs)
    return out
